# revision 13
# baseline (speedup 1.0000x reference)
"""Trainium2 Bass kernel for the attention-gate block (sample-major DMA).

Math (per sample n, after folding BN into the convs):
  X     = x[n, :, ::2, ::2].reshape(C, 4)                 # C=512, L=4
  act_k = relu(Wk' @ X + bk')            k=0,1,2          # D=64 each
  S     = act0^T act1  (4x4);  P = softmax_rows(S)
  Z     = P @ act2^T  (4x64)
  Y     = W4' @ Z^T + b4'                                  # (512, 4)
  out[n,c,h,w] = x[n,c,h,w] + Y[c,h]                       # broadcast over w

Device mapping (per core, 256 samples, blocks of 128):
  - SAMPLE-MAJOR DMA: partition = sample, so each partition line moves one
    8KB-contiguous chunk of a sample's row (4 chunk DMAs per block each
    way) -> line-rate HBM instead of 256B-packet descriptors.
  - the ::2,::2 gather + channel-major layout for the convs comes from 16
    PE transposes per block ([n,c]->[c,n] per (c-chunk k, position l)),
    evacuated to bf16 SBUF tiles; all conv/attention matmuls run in bf16.
  - attention: per 32-sample sub, one [64]x[128,128] gram matmul whose
    mask is tile(eye(32),(4,4)) under the (l-major, n-minor) column
    order; masked exp on ACT+DVE; denominators via a ones-column matmul,
    spread onto partitions as [sample, l] via 4 tiny outer-product
    matmuls per sub; ONE reciprocal for the whole block.
  - GEMM2 contracts over d with a bias row folded in (K=65), producing
    [c-chunk, sample] tiles that are PE-transposed back to sample-major;
    the softmax normalization rides the evacuation as a per-partition
    tensor_scalar multiply.
  - residual: one scalar_tensor_tensor per chunk adds Y (w-broadcast via
    a step-0 AP) into the fp32 x tile in place; stores go out on the
    scalar HWDGE queue while loads use the sync queue.
"""

import sys

for _p in ("/opt/trn_rl_repo",):
    if _p not in sys.path:
        sys.path.insert(0, _p)

import numpy as np
import ml_dtypes

import concourse.mybir as mybir
from concourse import bacc, tile

EPS = 1e-5
N_TOTAL, C, D, HH, WW = 2048, 512, 64, 4, 4
NCORES = 8
NSH = N_TOTAL // NCORES  # 256 samples per core
BLK = 128                # samples per block (= partition dim)
SUB = 32                 # samples per attention subchunk
NCH = 4                  # c-chunks of 128 channels
SHIFT = -34.0            # constant exp shift; cancels in the normalization
F32 = mybir.dt.float32
BF16 = mybir.dt.bfloat16

_PROG_CACHE = {}


def build_program(nsh=NSH, blk=BLK, reps=1):
    key = (nsh, blk, reps)
    if key in _PROG_CACHE:
        return _PROG_CACHE[key]
    assert blk == 128 and nsh % blk == 0

    nc = bacc.Bacc("TRN2", target_bir_lowering=False, debug=False)
    AF = mybir.ActivationFunctionType
    ADD = mybir.AluOpType.add

    x_in = nc.dram_tensor("x", (nsh, C, HH, WW), F32, kind="ExternalInput")
    wq = nc.dram_tensor("wq", (128, NCH, D), BF16, kind="ExternalInput")
    wk = nc.dram_tensor("wk", (128, NCH, D), BF16, kind="ExternalInput")
    w2 = nc.dram_tensor("w2", (128, NCH, D), BF16, kind="ExternalInput")
    w4a = nc.dram_tensor("w4a", (D + 1, NCH, 128), BF16, kind="ExternalInput")
    bq = nc.dram_tensor("bq", (D, 1), F32, kind="ExternalInput")
    bk = nc.dram_tensor("bk", (D, 1), F32, kind="ExternalInput")
    b2 = nc.dram_tensor("b2", (1, D), BF16, kind="ExternalInput")
    msk = nc.dram_tensor("msk", (128, 128), BF16, kind="ExternalInput")
    idn = nc.dram_tensor("idn", (128, 128), F32, kind="ExternalInput")
    ey4 = nc.dram_tensor("ey4", (1, 16), BF16, kind="ExternalInput")
    out = nc.dram_tensor("out", (nsh, C, HH, WW), F32, kind="ExternalOutput")

    nblk = nsh // blk
    CH = 2048  # elems per c-chunk of an x row: 128 c * 16 hw

    with tile.TileContext(nc) as tc:
        with (
            tc.tile_pool(name="const", bufs=1) as cpool,
            tc.tile_pool(name="xc", bufs=10) as xpool,
            tc.tile_pool(name="xsT", bufs=9) as tpool,
            tc.tile_pool(name="work", bufs=4) as wpool,
            tc.tile_pool(name="att", bufs=6) as apool,
            tc.tile_pool(name="ynm", bufs=6) as ypool,
            tc.tile_pool(name="ps", bufs=3, space="PSUM") as psA,
            tc.tile_pool(name="psr", bufs=1, space="PSUM") as psR,
            tc.tile_pool(name="pst", bufs=2, space="PSUM") as psT,
            tc.tile_pool(name="psy", bufs=2, space="PSUM") as psY,
        ):
            wq_sb = cpool.tile([128, NCH, D], BF16)
            nc.sync.dma_start(wq_sb[:], wq[:])
            wk_sb = cpool.tile([128, NCH, D], BF16)
            nc.sync.dma_start(wk_sb[:], wk[:])
            w2_sb = cpool.tile([128, NCH, D], BF16)
            nc.sync.dma_start(w2_sb[:], w2[:])
            w4a_sb = cpool.tile([D + 1, NCH, 128], BF16)
            nc.sync.dma_start(w4a_sb[:], w4a[:])
            bq_sb = cpool.tile([D, 1], F32)
            nc.sync.dma_start(bq_sb[:], bq[:])
            bk_sb = cpool.tile([D, 1], F32)
            nc.sync.dma_start(bk_sb[:], bk[:])
            b2_sb = cpool.tile([1, D], BF16)
            nc.sync.dma_start(b2_sb[:], b2[:])
            msk_sb = cpool.tile([128, 128], BF16)
            nc.sync.dma_start(msk_sb[:], msk[:])
            idn_sb = cpool.tile([128, 128], F32)
            nc.sync.dma_start(idn_sb[:], idn[:])
            ey4_sb = cpool.tile([1, 16], BF16)
            nc.sync.dma_start(ey4_sb[:], ey4[:])
            ones_r = cpool.tile([1, 128], BF16)
            nc.vector.memset(ones_r[:], 1.0)
            ones_c = cpool.tile([128, 1], BF16)
            nc.vector.memset(ones_c[:], 1.0)
            shift_sb = cpool.tile([128, 1], F32)
            nc.vector.memset(shift_sb[:], SHIFT)

            # sample-major views: one 8KB-contiguous run per (sample, chunk)
            xv = x_in[:].rearrange("(b n) c h w -> b n (c h w)", n=blk)
            ov = out[:].rearrange("(b n) c h w -> b n (c h w)", n=blk)

            for b in [b for _ in range(reps) for b in range(nblk)]:
                # ---- load x chunks (sample-major, line-rate) ----
                x_c = []
                for k in range(NCH):
                    xt = xpool.tile([128, CH], F32, tag="xc")
                    nc.sync.dma_start(xt[:], xv[b, :, k * CH:(k + 1) * CH])
                    x_c.append(xt)

                # ---- transpose the ::2,::2 picks to channel-major bf16 ----
                # xsT[k] cols are (n-major, l-minor): [128c, 128n, 4l]
                xsT = []
                for k in range(NCH):
                    xst = tpool.tile([128, 128, 4], BF16, tag="xsT")
                    xcv = x_c[k][:].rearrange("p (c h w) -> p c h w", h=4, w=4)
                    ps_t = psT.tile([128, 4, 128], F32, tag="pst")
                    for l in range(4):
                        hp, wp = (l // 2) * 2, (l % 2) * 2
                        nc.tensor.transpose(ps_t[:, l], xcv[:, :, hp, wp],
                                            idn_sb[:])
                    xtv = xst[:].rearrange("p n l -> p l n")
                    if k % 2 == 0:
                        nc.scalar.activation(xtv, ps_t[:], AF.Copy)
                    else:
                        nc.vector.tensor_copy(xtv, ps_t[:])
                    xsT.append(xst)

                # ---- GEMM1: q and k over 4 c-chunks ----
                ps_q = psA.tile([D, 512], F32, tag="ps")
                ps_k = psA.tile([D, 512], F32, tag="ps")
                for k in range(NCH):
                    xf = xsT[k][:].rearrange("p n l -> p (n l)")
                    nc.tensor.matmul(ps_q[:], lhsT=wq_sb[:, k], rhs=xf,
                                     start=(k == 0), stop=(k == 3))
                for k in range(NCH):
                    xf = xsT[k][:].rearrange("p n l -> p (n l)")
                    nc.tensor.matmul(ps_k[:], lhsT=wk_sb[:, k], rhs=xf,
                                     start=(k == 0), stop=(k == 3))
                a_q = wpool.tile([D, 512], BF16, tag="aq")
                nc.scalar.activation(a_q[:], ps_q[:], AF.Relu, bias=bq_sb[:])
                a_k = wpool.tile([D, 512], BF16, tag="ak")
                nc.scalar.activation(a_k[:], ps_k[:], AF.Relu, bias=bk_sb[:])

                # ---- attention per 32-sample sub ----
                # z_all rows 0..63 hold unnormalized P@V; row 64 holds the
                # softmax denominator per column, so GEMM2's K=65 contraction
                # yields W4@z + b4*d, and the 1/d evacuation scale recovers
                # W4@(z/d) + b4.
                z_all = apool.tile([D + 1, 4, 4, SUB], BF16, tag="z")  # (p, l, s, n)
                r_ps = psR.tile([128, 4], F32, tag="psr")
                for s in range(4):
                    ps_v = psA.tile([128, D], F32, tag="ps")
                    for k in range(NCH):
                        xl = (xsT[k][:, s * SUB:(s + 1) * SUB, :]
                              .rearrange("p n l -> p (n l)"))
                        nc.tensor.matmul(ps_v[:], lhsT=xl, rhs=w2_sb[:, k],
                                         start=(k == 0), stop=False)
                    nc.tensor.matmul(ps_v[:], lhsT=ones_r[:], rhs=b2_sb[:],
                                     start=False, stop=True)
                    a2t = apool.tile([128, D], BF16, tag="a2t")
                    nc.scalar.activation(a2t[:], ps_v[:], AF.Relu)

                    aks = a_k[:, s * 128:(s + 1) * 128]
                    aqs = a_q[:, s * 128:(s + 1) * 128]
                    ps_g = psA.tile([128, 128], F32, tag="ps")
                    nc.tensor.matmul(ps_g[:], lhsT=aks, rhs=aqs,
                                     start=True, stop=True)
                    e_t = apool.tile([128, 128], BF16, tag="e")
                    nc.scalar.activation(e_t[:], ps_g[:], AF.Exp,
                                         bias=shift_sb[:])
                    p0 = apool.tile([128, 128], BF16, tag="p0")
                    nc.vector.tensor_mul(p0[:], e_t[:], msk_sb[:])

                    # z rows 0..63 and the denominator row 64 share one
                    # PSUM tile -> a single evacuation per sub
                    ps_zd = psA.tile([D + 1, 128], F32, tag="ps")
                    nc.tensor.matmul(ps_zd[0:D, :], lhsT=a2t[:], rhs=p0[:],
                                     start=True, stop=True)
                    nc.tensor.matmul(ps_zd[D:D + 1, :], lhsT=ones_c[:],
                                     rhs=p0[:], start=True, stop=True,
                                     skip_group_check=True)
                    zdst = z_all[:, :, s, :].rearrange("p l n -> p n l")
                    zsrc = ps_zd[:].rearrange("p (n l) -> p n l", l=4)
                    if s % 2 == 0:
                        nc.scalar.activation(zdst, zsrc, AF.Copy)
                    else:
                        nc.vector.tensor_copy(zdst, zsrc)

                # denominators for the whole block as a base-0 row, spread
                # onto partitions as [sample, l] via 4 rank-1 matmuls
                d_blk = wpool.tile([1, 4, 128], BF16, tag="dblk")
                nc.scalar.activation(
                    d_blk[:], z_all[D:D + 1].rearrange("p l s n -> p l (s n)"),
                    AF.Copy)
                for l in range(4):
                    nc.tensor.matmul(
                        r_ps[:], lhsT=d_blk[:, l],
                        rhs=ey4_sb[0:1, l * 4:(l + 1) * 4],
                        start=(l == 0), stop=(l == 3),
                    )
                r_nm = wpool.tile([128, 4], F32, tag="r")
                nc.vector.reciprocal(r_nm[:], r_ps[:])

                # ---- GEMM2: z slice stationary -> sample-major Y directly,
                # softmax normalization folded into the evacuation scale ----
                w4f = w4a_sb[:].rearrange("p k c -> p (k c)")
                y_all = ypool.tile([128, 512, 4], BF16, tag="y")
                for h in range(4):
                    ps_y = psY.tile([128, 512], F32, tag="psy")
                    nc.tensor.matmul(
                        ps_y[:], lhsT=z_all[:, h].rearrange("p s n -> p (s n)"),
                        rhs=w4f, start=True, stop=True)
                    if h % 2 == 0:
                        nc.vector.tensor_scalar_mul(
                            y_all[:, :, h], ps_y[:], r_nm[:, h:h + 1])
                    else:
                        nc.scalar.activation(
                            y_all[:, :, h], ps_y[:], AF.Copy,
                            scale=r_nm[:, h:h + 1])

                # ---- residual add with w-broadcast; store each chunk ----
                for k in range(NCH):
                    xc4 = x_c[k][:].rearrange("p (c h w) -> p c h w", h=4, w=4)
                    ynb = (y_all[:, k * 128:(k + 1) * 128, :]
                           .unsqueeze(3).broadcast_to((128, 128, 4, 4)))
                    eng = nc.vector if k < 3 else nc.gpsimd
                    eng.tensor_add(xc4[:], ynb, xc4[:])
                    nc.scalar.dma_start(ov[b, :, k * CH:(k + 1) * CH], x_c[k][:])

    nc.compile()
    _PROG_CACHE[key] = nc
    return nc


def prep_params(W123, b123, g123, be123, m123, v123, W4, b4, g4, be4, m4, v4):
    """Fold BN into the convs; cast to bf16 in the natural c order."""
    f32, bf = np.float32, ml_dtypes.bfloat16
    s123 = (g123 / np.sqrt(v123 + EPS)).astype(f32)            # (3, D)
    Wf = (W123 * s123[:, :, None]).astype(f32)                 # (3, D, C)
    bf123 = ((b123 - m123) * s123 + be123).astype(f32)         # (3, D)
    s4 = (g4 / np.sqrt(v4 + EPS)).astype(f32)                  # (C,)
    W4f = (W4 * s4[:, None]).astype(f32)                       # (C, D)
    b4f = ((b4 - m4) * s4 + be4).astype(f32)                   # (C,)

    def chunks(wt):  # (C, D) -> (128, NCH, D)
        return np.ascontiguousarray(
            wt.reshape(NCH, 128, D).transpose(1, 0, 2)).astype(bf)

    w4a = np.concatenate([W4f.T, b4f[None, :]], axis=0)        # (65, C)
    w4a = np.ascontiguousarray(
        w4a.reshape(D + 1, NCH, 128)).astype(bf)
    msk = np.kron(np.eye(SUB, dtype=f32), np.ones((4, 4), f32)).astype(bf)
    ey4 = np.eye(4, dtype=f32).reshape(1, 16).astype(bf)
    return dict(
        wq=chunks(Wf[0].T), wk=chunks(Wf[1].T), w2=chunks(Wf[2].T),
        w4a=w4a,
        bq=np.ascontiguousarray(bf123[0][:, None]).astype(f32),
        bk=np.ascontiguousarray(bf123[1][:, None]).astype(f32),
        b2=np.ascontiguousarray(bf123[2][None, :]).astype(bf),
        msk=msk, idn=np.eye(128, dtype=f32), ey4=ey4,
    )


def _run(inputs, trace=False, **spmd_kwargs):
    from concourse.bass_utils import run_bass_kernel_spmd

    x = np.ascontiguousarray(np.asarray(inputs["x"], dtype=np.float32))
    params = prep_params(**{k: np.asarray(v, np.float64)
                            for k, v in inputs.items() if k != "x"})
    nc = build_program()
    in_maps = [
        {"x": x[i * NSH:(i + 1) * NSH], **params} for i in range(NCORES)
    ]
    res = run_bass_kernel_spmd(
        nc, in_maps, list(range(NCORES)), trace=trace, **spmd_kwargs
    )
    outs = np.concatenate(
        [np.asarray(res.results[i]["out"]) for i in range(NCORES)], axis=0
    )
    return outs, res


def kernel(**inputs):
    outs, _ = _run(inputs)
    return outs


# revision 14
# speedup vs baseline: 1.1222x; 1.1222x over previous
"""Trainium2 Bass kernel for the attention-gate block (sample-major DMA).

Math (per sample n, after folding BN into the convs):
  X     = x[n, :, ::2, ::2].reshape(C, 4)                 # C=512, L=4
  act_k = relu(Wk' @ X + bk')            k=0,1,2          # D=64 each
  S     = act0^T act1  (4x4);  P = softmax_rows(S)
  Z     = P @ act2^T  (4x64)
  Y     = W4' @ Z^T + b4'                                  # (512, 4)
  out[n,c,h,w] = x[n,c,h,w] + Y[c,h]                       # broadcast over w

Device mapping (per core, 256 samples, blocks of 128):
  - SAMPLE-MAJOR DMA: partition = sample, so each partition line moves one
    8KB-contiguous chunk of a sample's row (4 chunk DMAs per block each
    way) -> line-rate HBM instead of 256B-packet descriptors.
  - the ::2,::2 gather + channel-major layout for the convs comes from 16
    PE transposes per block ([n,c]->[c,n] per (c-chunk k, position l)),
    evacuated to bf16 SBUF tiles; all conv/attention matmuls run in bf16.
  - attention: per 32-sample sub, one [64]x[128,128] gram matmul whose
    mask is tile(eye(32),(4,4)) under the (l-major, n-minor) column
    order; masked exp on ACT+DVE; denominators via a ones-column matmul,
    spread onto partitions as [sample, l] via 4 tiny outer-product
    matmuls per sub; ONE reciprocal for the whole block.
  - GEMM2 contracts over d with a bias row folded in (K=65), producing
    [c-chunk, sample] tiles that are PE-transposed back to sample-major;
    the softmax normalization rides the evacuation as a per-partition
    tensor_scalar multiply.
  - residual: one scalar_tensor_tensor per chunk adds Y (w-broadcast via
    a step-0 AP) into the fp32 x tile in place; stores go out on the
    scalar HWDGE queue while loads use the sync queue.
"""

import sys

for _p in ("/opt/trn_rl_repo",):
    if _p not in sys.path:
        sys.path.insert(0, _p)

import numpy as np
import ml_dtypes

import concourse.mybir as mybir
from concourse import bacc, tile

EPS = 1e-5
N_TOTAL, C, D, HH, WW = 2048, 512, 64, 4, 4
NCORES = 8
NSH = N_TOTAL // NCORES  # 256 samples per core
BLK = 128                # samples per block (= partition dim)
SUB = 32                 # samples per attention subchunk
NCH = 4                  # c-chunks of 128 channels
SHIFT = -34.0            # constant exp shift; cancels in the normalization
F32 = mybir.dt.float32
BF16 = mybir.dt.bfloat16

_PROG_CACHE = {}


def build_program(nsh=NSH, blk=BLK, reps=1):
    key = (nsh, blk, reps)
    if key in _PROG_CACHE:
        return _PROG_CACHE[key]
    assert blk == 128 and nsh % blk == 0

    nc = bacc.Bacc("TRN2", target_bir_lowering=False, debug=False)
    AF = mybir.ActivationFunctionType
    ADD = mybir.AluOpType.add

    x_in = nc.dram_tensor("x", (nsh, C, HH, WW), F32, kind="ExternalInput")
    wq = nc.dram_tensor("wq", (128, NCH, D), BF16, kind="ExternalInput")
    wk = nc.dram_tensor("wk", (128, NCH, D), BF16, kind="ExternalInput")
    w2 = nc.dram_tensor("w2", (128, NCH, D), BF16, kind="ExternalInput")
    w4a = nc.dram_tensor("w4a", (D + 1, NCH, 128), BF16, kind="ExternalInput")
    bq = nc.dram_tensor("bq", (D, 1), F32, kind="ExternalInput")
    bk = nc.dram_tensor("bk", (D, 1), F32, kind="ExternalInput")
    b2 = nc.dram_tensor("b2", (1, D), BF16, kind="ExternalInput")
    msk = nc.dram_tensor("msk", (128, 128), BF16, kind="ExternalInput")
    idn = nc.dram_tensor("idn", (128, 128), F32, kind="ExternalInput")
    ey4 = nc.dram_tensor("ey4", (1, 16), BF16, kind="ExternalInput")
    out = nc.dram_tensor("out", (nsh, C, HH, WW), F32, kind="ExternalOutput")

    nblk = nsh // blk
    CH = 2048  # elems per c-chunk of an x row: 128 c * 16 hw

    with tile.TileContext(nc) as tc:
        with (
            tc.tile_pool(name="const", bufs=1) as cpool,
            tc.tile_pool(name="xc", bufs=10) as xpool,
            tc.tile_pool(name="xsT", bufs=9) as tpool,
            tc.tile_pool(name="work", bufs=4) as wpool,
            tc.tile_pool(name="att", bufs=6) as apool,
            tc.tile_pool(name="ynm", bufs=6) as ypool,
            tc.tile_pool(name="ps", bufs=3, space="PSUM") as psA,
            tc.tile_pool(name="psr", bufs=1, space="PSUM") as psR,
            tc.tile_pool(name="pst", bufs=2, space="PSUM") as psT,
            tc.tile_pool(name="psy", bufs=2, space="PSUM") as psY,
        ):
            wq_sb = cpool.tile([128, NCH, D], BF16)
            nc.scalar.dma_start(wq_sb[:], wq[:])
            wk_sb = cpool.tile([128, NCH, D], BF16)
            nc.scalar.dma_start(wk_sb[:], wk[:])
            w2_sb = cpool.tile([128, NCH, D], BF16)
            nc.scalar.dma_start(w2_sb[:], w2[:])
            w4a_sb = cpool.tile([D + 1, NCH, 128], BF16)
            nc.scalar.dma_start(w4a_sb[:], w4a[:])
            bq_sb = cpool.tile([D, 1], F32)
            nc.scalar.dma_start(bq_sb[:], bq[:])
            bk_sb = cpool.tile([D, 1], F32)
            nc.scalar.dma_start(bk_sb[:], bk[:])
            b2_sb = cpool.tile([1, D], BF16)
            nc.scalar.dma_start(b2_sb[:], b2[:])
            msk_sb = cpool.tile([128, 128], BF16)
            nc.scalar.dma_start(msk_sb[:], msk[:])
            idn_sb = cpool.tile([128, 128], F32)
            nc.scalar.dma_start(idn_sb[:], idn[:])
            ey4_sb = cpool.tile([1, 16], BF16)
            nc.scalar.dma_start(ey4_sb[:], ey4[:])
            ones_r = cpool.tile([1, 128], BF16)
            nc.vector.memset(ones_r[:], 1.0)
            ones_c = cpool.tile([128, 1], BF16)
            nc.vector.memset(ones_c[:], 1.0)
            shift_sb = cpool.tile([128, 1], F32)
            nc.vector.memset(shift_sb[:], SHIFT)

            # sample-major views: one 8KB-contiguous run per (sample, chunk)
            xv = x_in[:].rearrange("(b n) c h w -> b n (c h w)", n=blk)
            ov = out[:].rearrange("(b n) c h w -> b n (c h w)", n=blk)

            blocks = [b for _ in range(reps) for b in range(nblk)]
            NB = len(blocks)
            # phase-major emission: engines are strict FIFO, so a later
            # block's early ops must not queue behind an earlier block's
            # late ops.  All bulk DMA rides the sync ring (loads then
            # stores); consts went out on the scalar ring above.
            st_xc = [None] * NB
            st_xsT = [None] * NB
            st_aqk = [None] * NB
            st_z = [None] * NB
            st_r = [None] * NB
            st_y = [None] * NB

            for i, b in enumerate(blocks):
                x_c = []
                for k in range(NCH):
                    xt = xpool.tile([128, CH], F32, tag="xc")
                    nc.sync.dma_start(xt[:], xv[b, :, k * CH:(k + 1) * CH])
                    x_c.append(xt)
                st_xc[i] = x_c

            for i in range(NB):
                xsT = []
                for k in range(NCH):
                    xst = tpool.tile([128, 128, 4], BF16, tag="xsT")
                    xcv = st_xc[i][k][:].rearrange("p (c h w) -> p c h w",
                                                   h=4, w=4)
                    ps_t = psT.tile([128, 4, 128], F32, tag="pst")
                    for l in range(4):
                        hp, wp = (l // 2) * 2, (l % 2) * 2
                        nc.tensor.transpose(ps_t[:, l], xcv[:, :, hp, wp],
                                            idn_sb[:])
                    xtv = xst[:].rearrange("p n l -> p l n")
                    if k % 2 == 0:
                        nc.scalar.activation(xtv, ps_t[:], AF.Copy)
                    else:
                        nc.vector.tensor_copy(xtv, ps_t[:])
                    xsT.append(xst)
                st_xsT[i] = xsT

            for i in range(NB):
                xsT = st_xsT[i]
                ps_q = psA.tile([D, 512], F32, tag="ps")
                ps_k = psA.tile([D, 512], F32, tag="ps")
                for k in range(NCH):
                    xf = xsT[k][:].rearrange("p n l -> p (n l)")
                    nc.tensor.matmul(ps_q[:], lhsT=wq_sb[:, k], rhs=xf,
                                     start=(k == 0), stop=(k == 3))
                for k in range(NCH):
                    xf = xsT[k][:].rearrange("p n l -> p (n l)")
                    nc.tensor.matmul(ps_k[:], lhsT=wk_sb[:, k], rhs=xf,
                                     start=(k == 0), stop=(k == 3))
                a_q = wpool.tile([D, 512], BF16, tag="aq")
                nc.scalar.activation(a_q[:], ps_q[:], AF.Relu, bias=bq_sb[:])
                a_k = wpool.tile([D, 512], BF16, tag="ak")
                nc.scalar.activation(a_k[:], ps_k[:], AF.Relu, bias=bk_sb[:])
                st_aqk[i] = (a_q, a_k)

            for i in range(NB):
                xsT = st_xsT[i]
                a_q, a_k = st_aqk[i]
                z_all = apool.tile([D + 1, 4, 4, SUB], BF16, tag="z")
                for s in range(4):
                    ps_v = psA.tile([128, D], F32, tag="ps")
                    for k in range(NCH):
                        xl = (xsT[k][:, s * SUB:(s + 1) * SUB, :]
                              .rearrange("p n l -> p (n l)"))
                        nc.tensor.matmul(ps_v[:], lhsT=xl, rhs=w2_sb[:, k],
                                         start=(k == 0), stop=False)
                    nc.tensor.matmul(ps_v[:], lhsT=ones_r[:], rhs=b2_sb[:],
                                     start=False, stop=True)
                    a2t = apool.tile([128, D], BF16, tag="a2t")
                    nc.scalar.activation(a2t[:], ps_v[:], AF.Relu)

                    aks = a_k[:, s * 128:(s + 1) * 128]
                    aqs = a_q[:, s * 128:(s + 1) * 128]
                    ps_g = psA.tile([128, 128], F32, tag="ps")
                    nc.tensor.matmul(ps_g[:], lhsT=aks, rhs=aqs,
                                     start=True, stop=True)
                    e_t = apool.tile([128, 128], BF16, tag="e")
                    nc.scalar.activation(e_t[:], ps_g[:], AF.Exp,
                                         bias=shift_sb[:])
                    p0 = apool.tile([128, 128], BF16, tag="p0")
                    nc.vector.tensor_mul(p0[:], e_t[:], msk_sb[:])

                    ps_zd = psA.tile([D + 1, 128], F32, tag="ps")
                    nc.tensor.matmul(ps_zd[0:D, :], lhsT=a2t[:], rhs=p0[:],
                                     start=True, stop=True)
                    nc.tensor.matmul(ps_zd[D:D + 1, :], lhsT=ones_c[:],
                                     rhs=p0[:], start=True, stop=True,
                                     skip_group_check=True)
                    zdst = z_all[:, :, s, :].rearrange("p l n -> p n l")
                    zsrc = ps_zd[:].rearrange("p (n l) -> p n l", l=4)
                    if s % 2 == 0:
                        nc.scalar.activation(zdst, zsrc, AF.Copy)
                    else:
                        nc.vector.tensor_copy(zdst, zsrc)
                st_z[i] = z_all

            for i in range(NB):
                z_all = st_z[i]
                r_ps = psR.tile([128, 4], F32, tag="psr")
                d_blk = wpool.tile([1, 4, 128], BF16, tag="dblk")
                nc.scalar.activation(
                    d_blk[:],
                    z_all[D:D + 1].rearrange("p l s n -> p l (s n)"),
                    AF.Copy)
                for l in range(4):
                    nc.tensor.matmul(
                        r_ps[:], lhsT=d_blk[:, l],
                        rhs=ey4_sb[0:1, l * 4:(l + 1) * 4],
                        start=(l == 0), stop=(l == 3),
                    )
                r_nm = wpool.tile([128, 4], F32, tag="r")
                nc.vector.reciprocal(r_nm[:], r_ps[:])
                st_r[i] = r_nm

            w4f = w4a_sb[:].rearrange("p k c -> p (k c)")
            for i in range(NB):
                z_all, r_nm = st_z[i], st_r[i]
                y_all = ypool.tile([128, 512, 4], BF16, tag="y")
                for h in range(4):
                    ps_y = psY.tile([128, 512], F32, tag="psy")
                    nc.tensor.matmul(
                        ps_y[:],
                        lhsT=z_all[:, h].rearrange("p s n -> p (s n)"),
                        rhs=w4f, start=True, stop=True)
                    if h % 2 == 0:
                        nc.vector.tensor_scalar_mul(
                            y_all[:, :, h], ps_y[:], r_nm[:, h:h + 1])
                    else:
                        nc.scalar.activation(
                            y_all[:, :, h], ps_y[:], AF.Copy,
                            scale=r_nm[:, h:h + 1])
                st_y[i] = y_all

            for i, b in enumerate(blocks):
                x_c, y_all = st_xc[i], st_y[i]
                for k in range(NCH):
                    xc4 = x_c[k][:].rearrange("p (c h w) -> p c h w",
                                              h=4, w=4)
                    ynb = (y_all[:, k * 128:(k + 1) * 128, :]
                           .unsqueeze(3).broadcast_to((128, 128, 4, 4)))
                    eng = nc.vector if k < 3 else nc.gpsimd
                    eng.tensor_add(xc4[:], ynb, xc4[:])
                    nc.sync.dma_start(ov[b, :, k * CH:(k + 1) * CH],
                                      x_c[k][:])

    nc.compile()
    _PROG_CACHE[key] = nc
    return nc


def prep_params(W123, b123, g123, be123, m123, v123, W4, b4, g4, be4, m4, v4):
    """Fold BN into the convs; cast to bf16 in the natural c order."""
    f32, bf = np.float32, ml_dtypes.bfloat16
    s123 = (g123 / np.sqrt(v123 + EPS)).astype(f32)            # (3, D)
    Wf = (W123 * s123[:, :, None]).astype(f32)                 # (3, D, C)
    bf123 = ((b123 - m123) * s123 + be123).astype(f32)         # (3, D)
    s4 = (g4 / np.sqrt(v4 + EPS)).astype(f32)                  # (C,)
    W4f = (W4 * s4[:, None]).astype(f32)                       # (C, D)
    b4f = ((b4 - m4) * s4 + be4).astype(f32)                   # (C,)

    def chunks(wt):  # (C, D) -> (128, NCH, D)
        return np.ascontiguousarray(
            wt.reshape(NCH, 128, D).transpose(1, 0, 2)).astype(bf)

    w4a = np.concatenate([W4f.T, b4f[None, :]], axis=0)        # (65, C)
    w4a = np.ascontiguousarray(
        w4a.reshape(D + 1, NCH, 128)).astype(bf)
    msk = np.kron(np.eye(SUB, dtype=f32), np.ones((4, 4), f32)).astype(bf)
    ey4 = np.eye(4, dtype=f32).reshape(1, 16).astype(bf)
    return dict(
        wq=chunks(Wf[0].T), wk=chunks(Wf[1].T), w2=chunks(Wf[2].T),
        w4a=w4a,
        bq=np.ascontiguousarray(bf123[0][:, None]).astype(f32),
        bk=np.ascontiguousarray(bf123[1][:, None]).astype(f32),
        b2=np.ascontiguousarray(bf123[2][None, :]).astype(bf),
        msk=msk, idn=np.eye(128, dtype=f32), ey4=ey4,
    )


def _run(inputs, trace=False, **spmd_kwargs):
    from concourse.bass_utils import run_bass_kernel_spmd

    x = np.ascontiguousarray(np.asarray(inputs["x"], dtype=np.float32))
    params = prep_params(**{k: np.asarray(v, np.float64)
                            for k, v in inputs.items() if k != "x"})
    nc = build_program()
    in_maps = [
        {"x": x[i * NSH:(i + 1) * NSH], **params} for i in range(NCORES)
    ]
    res = run_bass_kernel_spmd(
        nc, in_maps, list(range(NCORES)), trace=trace, **spmd_kwargs
    )
    outs = np.concatenate(
        [np.asarray(res.results[i]["out"]) for i in range(NCORES)], axis=0
    )
    return outs, res


def kernel(**inputs):
    outs, _ = _run(inputs)
    return outs
